# revision 44
# baseline (speedup 1.0000x reference)
"""Trainium2 Bass kernel for quantized causal self-attention.

Sharding: Megatron-style tensor parallelism over heads. 16 heads are split
across 8 NeuronCores (2 heads/core). Each core computes, for all 4 batches:
  - its QKV head-slice projection (x @ w_slice^T, int8 weights held exactly
    in bf16, quant scales folded into epilogues / host),
  - causal attention for its 2 heads (transposed-scores layout, exp without
    max-subtraction - scores are bounded ~5 for this model family),
  - a partial output projection against its column slice of w_proj.
The host sums the 8 partial projections, applies the commuting quant scales
(s_w_attn * s_w_proj), and adds the biases that commute out of the linear
ops (c_proj bias, and the v-bias term which passes through softmax-normalized
attention as a constant row).

Schedule: fully software-pipelined across batches. Every stage s of batch b
interleaves, in PE program order, work with no mutual dependencies:
  scores(b,h0,row s) | QK(b+1, group s) | scores(b,h1,row s) |
  att@v(b,h0,qb=s) | V(b+1, group s) | att@v(b,h1,qb=s) |
  y-transposes | c_proj(b, tb=s-1)
so the tensor engine never waits on the scalar-engine exp or the DVE drain
chains. The causal-mask multiply runs on the otherwise-idle Pool engine and
the c_proj drains alternate DVE/ACT. Partials are written in bf16.
"""

import numpy as np
import ml_dtypes

B, T, C, H, D = 4, 1024, 2048, 16, 128
NCORES = 8
HPC = H // NCORES          # heads per core = 2
CS = HPC * D               # per-core head feature slice = 256
BT = B * T                 # 4096 tokens
TB = T // 128              # 8 token blocks per batch
CCH = C // 128             # 16 contraction chunks

BF16 = ml_dtypes.bfloat16

_CACHE = {}


def _build_program(s_wa: float):
    import concourse.tile as tile
    from concourse import bacc, mybir
    from concourse.masks import make_identity, make_upper_triangular

    f32 = mybir.dt.float32
    bf16 = mybir.dt.bfloat16
    i8 = mybir.dt.int8
    AF = mybir.ActivationFunctionType
    inv_sqrt_d = 1.0 / float(np.sqrt(D))

    nc = bacc.Bacc("TRN2", target_bir_lowering=False, debug=False)

    # all partition-major so SBUF-shaped multi-chunk DMAs need no transpose;
    # wqk is chunk-major so 4-chunk slices are 4KB-contiguous per partition
    # and arrive in the order the chunk-major prologue pass consumes them
    xT = nc.dram_tensor("xT", [B, 128, CCH, T], bf16, kind="ExternalInput")
    # weights ship as int8 (they are exactly int-valued) and are cast to
    # bf16 by the SWDGE during the DMA - halves the critical HBM bytes
    wqk = nc.dram_tensor("wqk", [128, CCH, 4, 128], i8, kind="ExternalInput")
    wv = nc.dram_tensor("wv", [128, CCH, CS], i8, kind="ExternalInput")
    wp = nc.dram_tensor("wp", [HPC, 128, C], i8, kind="ExternalInput")
    bqk = nc.dram_tensor("bqk", [128, 4], f32, kind="ExternalInput")
    partial = nc.dram_tensor("partial", [BT, C], bf16, kind="ExternalOutput")

    with tile.TileContext(nc) as tc:
        with (
            tc.tile_pool(name="singles", bufs=1) as singles,
            tc.tile_pool(name="xpool", bufs=2) as xpool,
            tc.tile_pool(name="qkpool", bufs=3) as qkpool,
            tc.tile_pool(name="vpool", bufs=3) as vpool,
            tc.tile_pool(name="attpool", bufs=3) as attpool,
            tc.tile_pool(name="ytpool", bufs=4) as ytpool,
            tc.tile_pool(name="ypool", bufs=3) as ypool,
            tc.tile_pool(name="rlpool", bufs=4) as rlpool,
            tc.tile_pool(name="outpool", bufs=4) as outpool,
            # 5 banks shared round-robin by the QK / V / c_proj / score
            # matmul groups: a 5-deep ring keeps the PE ~4 groups ahead of
            # the slowest drain (the scalar-engine exp)
            tc.tile_pool(name="psbig", bufs=5, space="PSUM") as psbig,
            # att@v accumulators: freed right after the normalize-multiply
            tc.tile_pool(name="pssm", bufs=2, space="PSUM") as pssm,
            # y-transpose target (PE transpose must land in PSUM)
            tc.tile_pool(name="pstp", bufs=1, space="PSUM") as pstp,
        ):
            # weights / constants, resident for the whole kernel
            wqk_s = singles.tile([128, CCH, 4, 128], bf16, tag="wqk")
            wv_s = singles.tile([128, CCH, CS], bf16, tag="wv")
            wp_s = [singles.tile([128, C], bf16, tag=f"wp{h}", name=f"wp_s{h}")
                    for h in range(HPC)]
            bqk_s = singles.tile([128, 4], f32, tag="bqk")
            trimask = singles.tile([128, 128], bf16, tag="trimask")
            ident = singles.tile([128, 128], bf16, tag="ident")

            # ---- emitters --------------------------------------------------
            def emit_qk_group(xb, qkT, idx, order=2):
                # idx -> (ob, th); ob: 0 = q head0, 1 = q head1, 2 = k
                # head0, 3 = k head1; th: halves of the 1024 tokens.
                # order=1 walks th-major (prologue: matches DMA arrival)
                if order == 2:
                    ob, th = idx // 2, idx % 2
                else:
                    ob, th = idx % 4, idx // 4
                ps = psbig.tile([128, 512], f32, tag="ps", name="ps")
                for g in range(CCH):
                    nc.tensor.matmul(
                        ps[:],
                        wqk_s[:, g, ob, :],
                        xb[:, g, th * 512:(th + 1) * 512],
                        start=(g == 0),
                        stop=(g == CCH - 1),
                    )
                nc.vector.tensor_scalar(
                    qkT[:, ob, th * 512:(th + 1) * 512],
                    ps[:],
                    s_wa,
                    bqk_s[:, ob:ob + 1],
                    mybir.AluOpType.mult,
                    mybir.AluOpType.add,
                )

            def emit_v_group(xb, v_all, tb):
                ps = psbig.tile([128, CS], f32, tag="ps", name="ps")
                for g in range(CCH):
                    nc.tensor.matmul(
                        ps[:],
                        xb[:, g, tb * 128:(tb + 1) * 128],
                        wv_s[:, g, :],
                        start=(g == 0),
                        stop=(g == CCH - 1),
                    )
                nc.vector.tensor_copy(
                    v_all[:, tb, :, 0:D],
                    ps[:].rearrange("p (h d) -> p h d", h=HPC),
                )

            def emit_score_row(qkT, h, attT, kb):
                width = T - kb * 128
                off = 0
                while off < width:
                    w = min(512, width - off)
                    ps = psbig.tile([128, 512], f32, tag="ps", name="ps")
                    nc.tensor.matmul(
                        ps[:, 0:w],
                        qkT[:, 2 + h, kb * 128:(kb + 1) * 128],
                        qkT[:, h, kb * 128 + off:kb * 128 + off + w],
                    )
                    nc.scalar.activation(
                        attT[:, kb, off:off + w],
                        ps[:, 0:w],
                        AF.Exp,
                        scale=inv_sqrt_d,
                    )
                    off += w
                # causal mask on the diagonal block (multiplicative), on the
                # otherwise-idle Pool engine
                nc.gpsimd.tensor_mul(
                    attT[:, kb, 0:128], attT[:, kb, 0:128], trimask[:],
                )

            def emit_av(attT, v_all, h, qb):
                # cols 0:D = y accum, col D = row-sum (ones column of v)
                psy = pssm.tile([128, 256], f32, tag="psy", name="psy")
                for kb in range(qb + 1):
                    nc.tensor.matmul(
                        psy[:, 0:D + 1],
                        attT[:, kb, (qb - kb) * 128:(qb - kb) * 128 + 128],
                        v_all[:, kb, h, :],
                        start=(kb == 0),
                        stop=(kb == qb),
                    )
                rl = rlpool.tile([128, 1], f32, tag="rl", name="rl")
                nc.vector.reciprocal(rl[:], psy[:, D:D + 1])
                ysb = ypool.tile([128, 128], bf16, tag="ysb", name="ysb")
                nc.vector.tensor_scalar_mul(ysb[:], psy[:, 0:D], rl[:])
                return psy, ysb

            def emit_transposes(chs, yTs, qb, pe=False):
                # y-transposes ride the DMA xbar (SBUF->SBUF, off the PE);
                # the PE path is kept for the final block of the last batch
                # where the ~1.5us DMA latency would sit on the critical
                # epilogue chain
                if pe:
                    pstile = pstp.tile([128, 128], f32, tag="pst",
                                       name="pstile")
                    for h, (psy, ysb) in enumerate(chs):
                        pst = pstile[:, 64 * h:64 * h + 64].bitcast(bf16)
                        nc.tensor.transpose(pst, ysb[:], ident[:])
                        nc.vector.tensor_copy(
                            yTs[h][:, qb * 128:(qb + 1) * 128], pst)
                else:
                    for h, (psy, ysb) in enumerate(chs):
                        nc.sync.dma_start(
                            yTs[h][:, qb * 128:(qb + 1) * 128], ysb[:],
                            transpose=True)

            def emit_proj(t0, yTs, tb, obs=(0, 1, 2, 3), po=None):
                # ob 0/1 drain on DVE, ob 2/3 on ACT, so the engine queues
                # stay balanced. Each engine-pair's half of the wide po tile
                # ships as its own 256KB DMA (2KB-contiguous descriptors) as
                # soon as both its drains are in, so the two store receipts
                # overlap at the end of the kernel.
                if po is None:
                    po = outpool.tile([128, C], bf16, tag="po", name="po")
                rows = partial[t0 + tb * 128:t0 + (tb + 1) * 128]
                for ob in obs:
                    ps = psbig.tile([128, 512], f32, tag="ps", name="ps")
                    for h in range(HPC):
                        nc.tensor.matmul(
                            ps[:],
                            yTs[h][:, tb * 128:(tb + 1) * 128],
                            wp_s[h][:, ob * 512:(ob + 1) * 512],
                            start=(h == 0),
                            stop=(h == HPC - 1),
                        )
                    if ob < 2:
                        nc.vector.tensor_copy(
                            po[:, ob * 512:(ob + 1) * 512], ps[:])
                    else:
                        nc.scalar.copy(
                            po[:, ob * 512:(ob + 1) * 512], ps[:])
                    if ob == 1:
                        nc.sync.dma_start(rows[:, 0:1024], po[:, 0:1024])
                    elif ob == 3:
                        # other HWDGE ring, so the two halves' issue and
                        # completion receipts overlap
                        nc.scalar.dma_start(rows[:, 1024:2048],
                                            po[:, 1024:2048])
                return po

            # ---- prologue --------------------------------------------------
            # weights dispatch from the (idle) ACT hardware-DGE ring, x from
            # SP; both rings drain FIFO, so issue in consumption order. The
            # first QK group (ob=0) starts as soon as bqk + wqk[0] (ACT ring)
            # and the first 2-chunk slice of x (SP ring) have landed (~1MB),
            # with the per-chunk matmuls streaming behind the x DMAs.
            xbs = [None] * B
            xbs[0] = xpool.tile([128, CCH, T], bf16, tag="xb", name="xb")
            xbs[1] = xpool.tile([128, CCH, T], bf16, tag="xb", name="xb")
            nc.scalar.dma_start(bqk_s[:], bqk[:])
            # weights ride the SWDGE (Pool) queue with an int8->bf16 cast;
            # x batch 0 alternates across both HWDGE rings (SP + ACT, both
            # idle in the prologue) - th=0 halves first (what the
            # chunk-major pass eats), then th=1. Each ring's first ~8 DMAs
            # issue immediately; later ones are completion-paced, which is
            # exactly the priority order we want. wv/wp/xb1 are gated
            # behind real prologue data (below).
            for lo, hi in ((0, 2), (2, 4), (4, 8), (8, 12), (12, 16)):
                nc.gpsimd.dma_start(wqk_s[:, lo:hi], wqk[:, lo:hi])
            x0_slices = [(0, 1), (1, 2)] + [(2 * a, 2 * a + 2)
                                            for a in range(1, 8)]
            for th in range(2):
                for i, (lo, hi) in enumerate(x0_slices):
                    eng = nc.sync if i % 2 == 0 else nc.scalar
                    eng.dma_start(
                        xbs[0][:, lo:hi, th * 512:(th + 1) * 512],
                        xT[0, :, lo:hi, th * 512:(th + 1) * 512])
            # valid (1.0) where q >= k for the transposed [k, q] diag block
            make_upper_triangular(nc, trimask[:], val=1.0, diag=True)
            make_identity(nc, ident[:])

            qkTs = [None] * B
            v_alls = [None] * B

            def alloc_batch(b):
                qkTs[b] = qkpool.tile([128, 4, T], bf16, tag="qkT",
                                      name="qkT")
                v_alls[b] = vpool.tile([128, TB, HPC, D + 1], bf16, tag="v",
                                       name="v_all")
                nc.vector.memset(v_alls[b][:, :, :, D:D + 1], 1.0)

            with nc.named_scope("prologue"):
                alloc_batch(0)
                # th=0 chunk-major pass: all 4 ob accumulators advance one
                # chunk at a time, so the PE consumes x/wqk chunks in DMA
                # arrival order at ~the HBM line rate, starting as soon as
                # the first chunk lands instead of after the full 4MB
                pss = [psbig.tile([128, 512], f32, tag="ps", name="ps")
                       for _ in range(4)]
                for g in range(CCH):
                    for ob in range(4):
                        nc.tensor.matmul(
                            pss[ob][:],
                            wqk_s[:, g, ob, :],
                            xbs[0][:, g, 0:512],
                            start=(g == 0),
                            stop=(g == CCH - 1),
                        )
                for ob in range(4):
                    nc.vector.tensor_scalar(
                        qkTs[0][:, ob, 0:512],
                        pss[ob][:],
                        s_wa,
                        bqk_s[:, ob:ob + 1],
                        mybir.AluOpType.mult,
                        mybir.AluOpType.add,
                    )
                # bandwidth gates: a 1-element copy whose SOURCE is data the
                # prologue produces late (a qkT drain) makes the gated DMA
                # genuinely unready until then - the scheduler hoists
                # dependency-free gates like memsets, but not these. The
                # copied garbage is overwritten by the DMA itself.
                nc.vector.tensor_copy(wv_s[:, 0:1, 0:1],
                                      qkTs[0][:, 0:1, 0:1])
                for a in range(2):
                    nc.gpsimd.dma_start(wv_s[:, 8 * a:8 * a + 8, :],
                                        wv[:, 8 * a:8 * a + 8, :])
                nc.vector.tensor_copy(xbs[1][:, 0:1, 0:1],
                                      qkTs[0][:, 1:2, 0:1])
                for a in range(8):
                    nc.sync.dma_start(xbs[1][:, 2 * a:2 * a + 2, :],
                                      xT[1, :, 2 * a:2 * a + 2, :])
                emit_qk_group(xbs[0], qkTs[0], 1)   # (ob0, th1)
                emit_qk_group(xbs[0], qkTs[0], 3)   # (ob1, th1)
                for h in range(HPC):
                    # gated on the (ob0, th1) drain just above
                    nc.vector.tensor_copy(wp_s[h][:, 0:1],
                                          qkTs[0][:, 0:1, 512:513])
                for h in range(HPC):
                    nc.gpsimd.dma_start(wp_s[h][:], wp[h])
                emit_qk_group(xbs[0], qkTs[0], 5)   # (ob2, th1)
                emit_qk_group(xbs[0], qkTs[0], 7)   # (ob3, th1)
                for tb in range(TB):
                    emit_v_group(xbs[0], v_alls[0], tb)

            # ---- pipelined batches ----------------------------------------
            prev = None            # deferred (t0, yTs) projection hand-off
            for b in range(B):
                t0 = b * T
                attTs = [attpool.tile([128, TB, T], bf16, tag="attT",
                                      name=f"attT{h}") for h in range(HPC)]
                yTs = [ytpool.tile([128, T], bf16, tag="yT",
                                   name=f"yT{h}") for h in range(HPC)]
                if b + 1 < B:
                    alloc_batch(b + 1)
                if b + 2 < B:
                    xbs[b + 2] = xpool.tile([128, CCH, T], bf16, tag="xb",
                                            name="xb")

                with nc.named_scope(f"batch{b}"):
                    if b == 3:
                        # no next-batch QK/V filler here, so run the score
                        # rows one step ahead of the avs: row s+1's exp+mask
                        # chain hides under av(s)+proj(s-1) PE work
                        emit_score_row(qkTs[b], 0, attTs[0], 0)
                        emit_score_row(qkTs[b], 1, attTs[1], 0)
                        for s in range(TB):
                            if s + 1 < TB:
                                emit_score_row(qkTs[b], 0, attTs[0], s + 1)
                            if s == 0 and prev is not None:
                                emit_proj(*prev, TB - 1)
                            if s + 1 < TB:
                                emit_score_row(qkTs[b], 1, attTs[1], s + 1)
                            if s > 0:
                                po3 = emit_proj(t0, yTs, s - 1, obs=(2, 3))
                            ch0 = emit_av(attTs[0], v_alls[b], 0, s)
                            ch1 = emit_av(attTs[1], v_alls[b], 1, s)
                            if s > 0:
                                emit_proj(t0, yTs, s - 1, obs=(0, 1), po=po3)
                            emit_transposes((ch0, ch1), yTs, s,
                                            pe=(s == TB - 1))
                        # interleave the final drains DVE/ACT so both
                        # engines finish right after the last matmul
                        emit_proj(t0, yTs, TB - 1, obs=(0, 2, 1, 3))
                    else:
                        for s in range(TB):
                            emit_score_row(qkTs[b], 0, attTs[0], s)
                            emit_qk_group(xbs[b + 1], qkTs[b + 1], s)
                            emit_score_row(qkTs[b], 1, attTs[1], s)
                            if s == 0 and prev is not None:
                                # previous batch's last projection block
                                # lands here, where its y-transpose chain
                                # has surely drained
                                emit_proj(*prev, TB - 1)
                            ch0 = emit_av(attTs[0], v_alls[b], 0, s)
                            emit_v_group(xbs[b + 1], v_alls[b + 1], s)
                            ch1 = emit_av(attTs[1], v_alls[b], 1, s)
                            emit_transposes((ch0, ch1), yTs, s)
                            if s > 0:
                                emit_proj(t0, yTs, s - 1)
                            if b + 2 < B:
                                nc.sync.dma_start(
                                    xbs[b + 2][:, 2 * s:2 * s + 2, :],
                                    xT[b + 2, :, 2 * s:2 * s + 2, :])
                        prev = (t0, yTs)       # proj(b, 7) deferred to b+1

    nc.compile()
    return nc


def kernel(x, w_attn_q, s_w_attn, z_w_attn, b_attn_q, s_b_attn, z_b_attn,
           w_proj_q, s_w_proj, z_w_proj, b_proj_q, s_b_proj, z_b_proj):
    from concourse.bass_utils import run_bass_kernel_spmd

    x = np.asarray(x, np.float32)
    w_attn_q = np.asarray(w_attn_q)
    b_attn_q = np.asarray(b_attn_q)
    w_proj_q = np.asarray(w_proj_q)
    b_proj_q = np.asarray(b_proj_q)
    s_wa = float(s_w_attn)
    s_ba = float(s_b_attn)
    s_wp = float(s_w_proj)
    s_bp = float(s_b_proj)

    # integer-valued dequantized weights; z is 0 for this symmetric scheme
    # so they fit int8 exactly (device casts int8 -> bf16 during the DMA)
    wa_i32 = w_attn_q.astype(np.int32) - int(z_w_attn)
    wp_i32 = w_proj_q.astype(np.int32) - int(z_w_proj)
    assert wa_i32.min() >= -128 and wa_i32.max() <= 127
    assert wp_i32.min() >= -128 and wp_i32.max() <= 127
    wa_int = wa_i32.astype(np.float32)
    wp_int = wp_i32.astype(np.float32)
    wa_i8 = wa_i32.astype(np.int8)
    wp_i8 = wp_i32.astype(np.int8)
    ba_true = s_ba * (b_attn_q.astype(np.int32) - int(z_b_attn)).astype(np.float32)
    bp_true = s_bp * (b_proj_q.astype(np.int32) - int(z_b_proj)).astype(np.float32)

    xT_np = np.ascontiguousarray(
        np.swapaxes(x, 1, 2).reshape(B, CCH, 128, T).transpose(0, 2, 1, 3)
    ).astype(BF16)                                   # [B, 128, CCH, T]

    key = (s_wa,)
    if key not in _CACHE:
        _CACHE[key] = _build_program(s_wa)
    nc = _CACHE[key]

    in_maps = []
    for c in range(NCORES):
        r0 = c * CS                    # q rows for this core's heads
        wq = wa_i8[r0:r0 + CS]                     # [256, C]
        wk = wa_i8[C + r0:C + r0 + CS]
        wv_rows = wa_i8[2 * C + r0:2 * C + r0 + CS]
        wqk_np = np.ascontiguousarray(
            np.concatenate([wq, wk], axis=0).T       # [C, 512]
            .reshape(CCH, 128, 4, 128).transpose(1, 0, 2, 3)
        )                                            # [128, CCH, 4, 128] i8
        wv_np = np.ascontiguousarray(
            wv_rows.T.reshape(CCH, 128, CS).transpose(1, 0, 2)
        )                                            # [128, CCH, 256] i8
        wp_np = np.ascontiguousarray(
            wp_i8[:, r0:r0 + CS].T                   # [256, C]
        ).reshape(HPC, 128, C)
        bq = ba_true[r0:r0 + CS]
        bk = ba_true[C + r0:C + r0 + CS]
        bqk_np = np.ascontiguousarray(
            np.concatenate([bq, bk]).reshape(4, 128).T  # [128, 4]
        ).astype(np.float32)
        in_maps.append({
            "xT": xT_np,
            "wqk0b": wqk_np[:, 0:4].astype(BF16),
            "wqk": wqk_np,
            "wv": wv_np,
            "wp": wp_np,
            "bqk": bqk_np,
        })

    res = run_bass_kernel_spmd(nc, in_maps, core_ids=list(range(NCORES)))

    acc = np.zeros((BT, C), np.float64)
    for c in range(NCORES):
        acc += res.results[c]["partial"].astype(np.float64)
    # v and w_proj were used unscaled on device; apply the commuting scales
    # here. The v-bias passes through normalized attention as a constant row;
    # add it (and the c_proj bias) here, exactly, in fp64->fp32.
    bv_true = ba_true[2 * C:3 * C]
    bv_fold = (s_wp * (bv_true.astype(np.float64) @ wp_int.astype(np.float64).T))
    out = (s_wa * s_wp) * acc + bv_fold[None, :] + bp_true.astype(np.float64)[None, :]
    return out.reshape(B, T, C).astype(np.float32)



# revision 50
# speedup vs baseline: 1.1926x; 1.1926x over previous
"""Trainium2 Bass kernel for quantized causal self-attention.

Sharding: Megatron-style tensor parallelism over heads. 16 heads are split
across 8 NeuronCores (2 heads/core). Each core computes, for all 4 batches:
  - its QKV head-slice projection (x @ w_slice^T, int8 weights held exactly
    in bf16, quant scales folded into epilogues / host),
  - causal attention for its 2 heads (transposed-scores layout, exp without
    max-subtraction - scores are bounded ~5 for this model family),
  - a partial output projection against its column slice of w_proj.
The host sums the 8 partial projections, applies the commuting quant scales
(s_w_attn * s_w_proj), and adds the biases that commute out of the linear
ops (c_proj bias, and the v-bias term which passes through softmax-normalized
attention as a constant row).

Schedule: fully software-pipelined across batches. Every stage s of batch b
interleaves, in PE program order, work with no mutual dependencies:
  scores(b,h0,row s) | QK(b+1, group s) | scores(b,h1,row s) |
  att@v(b,h0,qb=s) | V(b+1, group s) | att@v(b,h1,qb=s) |
  y-transposes | c_proj(b, tb=s-1)
so the tensor engine never waits on the scalar-engine exp or the DVE drain
chains. The causal-mask multiply runs on the otherwise-idle Pool engine and
the c_proj drains alternate DVE/ACT. Partials are written in bf16.
"""

import numpy as np
import ml_dtypes

B, T, C, H, D = 4, 1024, 2048, 16, 128
NCORES = 8
HPC = H // NCORES          # heads per core = 2
CS = HPC * D               # per-core head feature slice = 256
BT = B * T                 # 4096 tokens
TB = T // 128              # 8 token blocks per batch
CCH = C // 128             # 16 contraction chunks

BF16 = ml_dtypes.bfloat16

_CACHE = {}


def _build_program(s_wa: float):
    import concourse.tile as tile
    from concourse import bacc, mybir
    from concourse.masks import make_identity, make_upper_triangular

    f32 = mybir.dt.float32
    bf16 = mybir.dt.bfloat16
    i8 = mybir.dt.int8
    AF = mybir.ActivationFunctionType
    inv_sqrt_d = 1.0 / float(np.sqrt(D))

    nc = bacc.Bacc("TRN2", target_bir_lowering=False, debug=False)

    # all partition-major so SBUF-shaped multi-chunk DMAs need no transpose;
    # wqk is chunk-major so 4-chunk slices are 4KB-contiguous per partition
    # and arrive in the order the chunk-major prologue pass consumes them
    xT = nc.dram_tensor("xT", [B, 128, CCH, T], bf16, kind="ExternalInput")
    # weights ship as int8 (they are exactly int-valued) and are cast to
    # bf16 by the SWDGE during the DMA - halves the critical HBM bytes
    wqk = nc.dram_tensor("wqk", [128, CCH, 4, 128], i8, kind="ExternalInput")
    wv = nc.dram_tensor("wv", [128, CCH, CS], i8, kind="ExternalInput")
    wp = nc.dram_tensor("wp", [HPC, 128, C], i8, kind="ExternalInput")
    bqk = nc.dram_tensor("bqk", [128, 4], f32, kind="ExternalInput")
    partial = nc.dram_tensor("partial", [BT, C], bf16, kind="ExternalOutput")

    with tile.TileContext(nc) as tc:
        with (
            tc.tile_pool(name="singles", bufs=1) as singles,
            tc.tile_pool(name="xpool", bufs=2) as xpool,
            tc.tile_pool(name="qkpool", bufs=3) as qkpool,
            tc.tile_pool(name="vpool", bufs=3) as vpool,
            tc.tile_pool(name="attpool", bufs=3) as attpool,
            tc.tile_pool(name="ytpool", bufs=4) as ytpool,
            tc.tile_pool(name="ypool", bufs=3) as ypool,
            tc.tile_pool(name="rlpool", bufs=4) as rlpool,
            tc.tile_pool(name="outpool", bufs=4) as outpool,
            # 5 banks shared round-robin by the QK / V / c_proj / score
            # matmul groups: a 5-deep ring keeps the PE ~4 groups ahead of
            # the slowest drain (the scalar-engine exp)
            tc.tile_pool(name="psbig", bufs=5, space="PSUM") as psbig,
            # att@v accumulators: freed right after the normalize-multiply
            tc.tile_pool(name="pssm", bufs=2, space="PSUM") as pssm,
            # y-transpose target (PE transpose must land in PSUM)
            tc.tile_pool(name="pstp", bufs=1, space="PSUM") as pstp,
        ):
            # weights / constants, resident for the whole kernel
            wqk_s = singles.tile([128, CCH, 4, 128], bf16, tag="wqk")
            wv_s = singles.tile([128, CCH, CS], bf16, tag="wv")
            wp_s = [singles.tile([128, C], bf16, tag=f"wp{h}", name=f"wp_s{h}")
                    for h in range(HPC)]
            bqk_s = singles.tile([128, 4], f32, tag="bqk")
            trimask = singles.tile([128, 128], bf16, tag="trimask")
            ident = singles.tile([128, 128], bf16, tag="ident")

            # ---- emitters --------------------------------------------------
            def emit_qk_group(xb, qkT, idx, order=2):
                # idx -> (ob, th); ob: 0 = q head0, 1 = q head1, 2 = k
                # head0, 3 = k head1; th: halves of the 1024 tokens.
                # order=1 walks th-major (prologue: matches DMA arrival)
                if order == 2:
                    ob, th = idx // 2, idx % 2
                else:
                    ob, th = idx % 4, idx // 4
                ps = psbig.tile([128, 512], f32, tag="ps", name="ps")
                for g in range(CCH):
                    nc.tensor.matmul(
                        ps[:],
                        wqk_s[:, g, ob, :],
                        xb[:, g, th * 512:(th + 1) * 512],
                        start=(g == 0),
                        stop=(g == CCH - 1),
                    )
                nc.vector.tensor_scalar(
                    qkT[:, ob, th * 512:(th + 1) * 512],
                    ps[:],
                    s_wa,
                    bqk_s[:, ob:ob + 1],
                    mybir.AluOpType.mult,
                    mybir.AluOpType.add,
                )

            def emit_v_group(xb, v_all, tb):
                ps = psbig.tile([128, CS], f32, tag="ps", name="ps")
                for g in range(CCH):
                    nc.tensor.matmul(
                        ps[:],
                        xb[:, g, tb * 128:(tb + 1) * 128],
                        wv_s[:, g, :],
                        start=(g == 0),
                        stop=(g == CCH - 1),
                    )
                nc.vector.tensor_copy(
                    v_all[:, tb, :, 0:D],
                    ps[:].rearrange("p (h d) -> p h d", h=HPC),
                )

            def emit_score_row(qkT, h, attT, kb):
                width = T - kb * 128
                off = 0
                while off < width:
                    w = min(512, width - off)
                    ps = psbig.tile([128, 512], f32, tag="ps", name="ps")
                    nc.tensor.matmul(
                        ps[:, 0:w],
                        qkT[:, 2 + h, kb * 128:(kb + 1) * 128],
                        qkT[:, h, kb * 128 + off:kb * 128 + off + w],
                    )
                    nc.scalar.activation(
                        attT[:, kb, off:off + w],
                        ps[:, 0:w],
                        AF.Exp,
                        scale=inv_sqrt_d,
                    )
                    off += w
                # causal mask on the diagonal block (multiplicative), on the
                # otherwise-idle Pool engine
                nc.gpsimd.tensor_mul(
                    attT[:, kb, 0:128], attT[:, kb, 0:128], trimask[:],
                )

            def emit_av(attT, v_all, h, qb):
                # cols 0:D = y accum, col D = row-sum (ones column of v)
                psy = pssm.tile([128, 256], f32, tag="psy", name="psy")
                for kb in range(qb + 1):
                    nc.tensor.matmul(
                        psy[:, 0:D + 1],
                        attT[:, kb, (qb - kb) * 128:(qb - kb) * 128 + 128],
                        v_all[:, kb, h, :],
                        start=(kb == 0),
                        stop=(kb == qb),
                    )
                rl = rlpool.tile([128, 1], f32, tag="rl", name="rl")
                nc.vector.reciprocal(rl[:], psy[:, D:D + 1])
                ysb = ypool.tile([128, 128], bf16, tag="ysb", name="ysb")
                nc.vector.tensor_scalar_mul(ysb[:], psy[:, 0:D], rl[:])
                return psy, ysb

            def emit_transposes(chs, yTs, qb, pe=True):
                # dedicated PSUM targets (both heads in one bank slot) so
                # psy recycles as soon as the normalize-multiply has read it
                pstile = pstp.tile([128, 128], f32, tag="pst", name="pstile")
                for h, (psy, ysb) in enumerate(chs):
                    pst = pstile[:, 64 * h:64 * h + 64].bitcast(bf16)
                    nc.tensor.transpose(pst, ysb[:], ident[:])
                    nc.vector.tensor_copy(
                        yTs[h][:, qb * 128:(qb + 1) * 128], pst)

            def emit_proj(t0, yTs, tb, obs=(0, 1, 2, 3), po=None):
                # ob 0/1 drain on DVE, ob 2/3 on ACT, so the engine queues
                # stay balanced. Each engine-pair's half of the wide po tile
                # ships as its own 256KB DMA (2KB-contiguous descriptors) as
                # soon as both its drains are in, so the two store receipts
                # overlap at the end of the kernel.
                if po is None:
                    po = outpool.tile([128, C], bf16, tag="po", name="po")
                rows = partial[t0 + tb * 128:t0 + (tb + 1) * 128]
                for ob in obs:
                    ps = psbig.tile([128, 512], f32, tag="ps", name="ps")
                    for h in range(HPC):
                        nc.tensor.matmul(
                            ps[:],
                            yTs[h][:, tb * 128:(tb + 1) * 128],
                            wp_s[h][:, ob * 512:(ob + 1) * 512],
                            start=(h == 0),
                            stop=(h == HPC - 1),
                        )
                    if ob < 2:
                        nc.vector.tensor_copy(
                            po[:, ob * 512:(ob + 1) * 512], ps[:])
                    else:
                        nc.scalar.copy(
                            po[:, ob * 512:(ob + 1) * 512], ps[:])
                    if ob == 1:
                        nc.sync.dma_start(rows[:, 0:1024], po[:, 0:1024])
                    elif ob == 3:
                        # other HWDGE ring, so the two halves' issue and
                        # completion receipts overlap
                        nc.scalar.dma_start(rows[:, 1024:2048],
                                            po[:, 1024:2048])
                return po

            # ---- prologue --------------------------------------------------
            # weights dispatch from the (idle) ACT hardware-DGE ring, x from
            # SP; both rings drain FIFO, so issue in consumption order. The
            # first QK group (ob=0) starts as soon as bqk + wqk[0] (ACT ring)
            # and the first 2-chunk slice of x (SP ring) have landed (~1MB),
            # with the per-chunk matmuls streaming behind the x DMAs.
            xbs = [None] * B
            xbs[0] = xpool.tile([128, CCH, T], bf16, tag="xb", name="xb")
            xbs[1] = xpool.tile([128, CCH, T], bf16, tag="xb", name="xb")
            nc.scalar.dma_start(bqk_s[:], bqk[:])
            # weights ride the SWDGE (Pool) queue with an int8->bf16 cast;
            # x batch 0 alternates across both HWDGE rings (SP + ACT, both
            # idle in the prologue) - th=0 halves first (what the
            # chunk-major pass eats), then th=1. Each ring's first ~8 DMAs
            # issue immediately; later ones are completion-paced, which is
            # exactly the priority order we want. wv/wp/xb1 are gated
            # behind real prologue data (below).
            for q in range(4):
                nc.gpsimd.dma_start(wqk_s[:, 4 * q:4 * q + 4],
                                    wqk[:, 4 * q:4 * q + 4])
            for th in range(2):
                for a in range(8):
                    eng = nc.sync if a % 2 == 0 else nc.scalar
                    eng.dma_start(
                        xbs[0][:, 2 * a:2 * a + 2, th * 512:(th + 1) * 512],
                        xT[0, :, 2 * a:2 * a + 2, th * 512:(th + 1) * 512])
            # valid (1.0) where q >= k for the transposed [k, q] diag block
            make_upper_triangular(nc, trimask[:], val=1.0, diag=True)
            make_identity(nc, ident[:])

            qkTs = [None] * B
            v_alls = [None] * B

            def alloc_batch(b):
                qkTs[b] = qkpool.tile([128, 4, T], bf16, tag="qkT",
                                      name="qkT")
                v_alls[b] = vpool.tile([128, TB, HPC, D + 1], bf16, tag="v",
                                       name="v_all")
                nc.vector.memset(v_alls[b][:, :, :, D:D + 1], 1.0)

            with nc.named_scope("prologue"):
                alloc_batch(0)
                # th=0 chunk-major pass: all 4 ob accumulators advance one
                # chunk at a time, so the PE consumes x/wqk chunks in DMA
                # arrival order at ~the HBM line rate, starting as soon as
                # the first chunk lands instead of after the full 4MB
                pss = [psbig.tile([128, 512], f32, tag="ps", name="ps")
                       for _ in range(4)]
                for g in range(CCH):
                    for ob in range(4):
                        nc.tensor.matmul(
                            pss[ob][:],
                            wqk_s[:, g, ob, :],
                            xbs[0][:, g, 0:512],
                            start=(g == 0),
                            stop=(g == CCH - 1),
                        )
                for ob in range(4):
                    nc.vector.tensor_scalar(
                        qkTs[0][:, ob, 0:512],
                        pss[ob][:],
                        s_wa,
                        bqk_s[:, ob:ob + 1],
                        mybir.AluOpType.mult,
                        mybir.AluOpType.add,
                    )
                # bandwidth gates: a 1-element copy whose SOURCE is data the
                # prologue produces late (a qkT drain) makes the gated DMA
                # genuinely unready until then - the scheduler hoists
                # dependency-free gates like memsets, but not these. The
                # copied garbage is overwritten by the DMA itself.
                nc.vector.tensor_copy(wv_s[:, 0:1, 0:1],
                                      qkTs[0][:, 0:1, 0:1])
                for a in range(2):
                    nc.gpsimd.dma_start(wv_s[:, 8 * a:8 * a + 8, :],
                                        wv[:, 8 * a:8 * a + 8, :])
                nc.vector.tensor_copy(xbs[1][:, 0:1, 0:1],
                                      qkTs[0][:, 1:2, 0:1])
                for a in range(8):
                    nc.sync.dma_start(xbs[1][:, 2 * a:2 * a + 2, :],
                                      xT[1, :, 2 * a:2 * a + 2, :])
                emit_qk_group(xbs[0], qkTs[0], 1)   # (ob0, th1)
                emit_qk_group(xbs[0], qkTs[0], 3)   # (ob1, th1)
                for h in range(HPC):
                    # gated on the (ob0, th1) drain just above
                    nc.vector.tensor_copy(wp_s[h][:, 0:1],
                                          qkTs[0][:, 0:1, 512:513])
                for h in range(HPC):
                    nc.gpsimd.dma_start(wp_s[h][:], wp[h])
                emit_qk_group(xbs[0], qkTs[0], 5)   # (ob2, th1)
                emit_qk_group(xbs[0], qkTs[0], 7)   # (ob3, th1)
                for tb in range(TB):
                    emit_v_group(xbs[0], v_alls[0], tb)

            # ---- pipelined batches ----------------------------------------
            prev = None            # deferred (t0, yTs) projection hand-off
            for b in range(B):
                t0 = b * T
                attTs = [attpool.tile([128, TB, T], bf16, tag="attT",
                                      name=f"attT{h}") for h in range(HPC)]
                yTs = [ytpool.tile([128, T], bf16, tag="yT",
                                   name=f"yT{h}") for h in range(HPC)]
                if b + 1 < B:
                    alloc_batch(b + 1)
                if b + 2 < B:
                    xbs[b + 2] = xpool.tile([128, CCH, T], bf16, tag="xb",
                                            name="xb")

                with nc.named_scope(f"batch{b}"):
                    if b == 3:
                        # no next-batch QK/V filler here, so run the score
                        # rows one step ahead of the avs: row s+1's exp+mask
                        # chain hides under av(s)+proj(s-1) PE work
                        emit_score_row(qkTs[b], 0, attTs[0], 0)
                        emit_score_row(qkTs[b], 1, attTs[1], 0)
                        for s in range(TB):
                            if s + 1 < TB:
                                emit_score_row(qkTs[b], 0, attTs[0], s + 1)
                            if s == 0 and prev is not None:
                                emit_proj(*prev, TB - 1)
                            if s + 1 < TB:
                                emit_score_row(qkTs[b], 1, attTs[1], s + 1)
                            if s > 0:
                                po3 = emit_proj(t0, yTs, s - 1, obs=(2, 3))
                            ch0 = emit_av(attTs[0], v_alls[b], 0, s)
                            ch1 = emit_av(attTs[1], v_alls[b], 1, s)
                            if s > 0:
                                emit_proj(t0, yTs, s - 1, obs=(0, 1), po=po3)
                            emit_transposes((ch0, ch1), yTs, s,
                                            pe=(s == TB - 1))
                        # interleave the final drains DVE/ACT so both
                        # engines finish right after the last matmul
                        emit_proj(t0, yTs, TB - 1, obs=(0, 2, 1, 3))
                    else:
                        for s in range(TB):
                            emit_score_row(qkTs[b], 0, attTs[0], s)
                            emit_qk_group(xbs[b + 1], qkTs[b + 1], s)
                            emit_score_row(qkTs[b], 1, attTs[1], s)
                            if s == 0 and prev is not None:
                                # previous batch's last projection block
                                # lands here, where its y-transpose chain
                                # has surely drained
                                emit_proj(*prev, TB - 1)
                            ch0 = emit_av(attTs[0], v_alls[b], 0, s)
                            emit_v_group(xbs[b + 1], v_alls[b + 1], s)
                            ch1 = emit_av(attTs[1], v_alls[b], 1, s)
                            emit_transposes((ch0, ch1), yTs, s)
                            if s > 0:
                                emit_proj(t0, yTs, s - 1)
                            if b + 2 < B:
                                nc.sync.dma_start(
                                    xbs[b + 2][:, 2 * s:2 * s + 2, :],
                                    xT[b + 2, :, 2 * s:2 * s + 2, :])
                        prev = (t0, yTs)       # proj(b, 7) deferred to b+1

    nc.compile()
    return nc


def kernel(x, w_attn_q, s_w_attn, z_w_attn, b_attn_q, s_b_attn, z_b_attn,
           w_proj_q, s_w_proj, z_w_proj, b_proj_q, s_b_proj, z_b_proj):
    from concourse.bass_utils import run_bass_kernel_spmd

    x = np.asarray(x, np.float32)
    w_attn_q = np.asarray(w_attn_q)
    b_attn_q = np.asarray(b_attn_q)
    w_proj_q = np.asarray(w_proj_q)
    b_proj_q = np.asarray(b_proj_q)
    s_wa = float(s_w_attn)
    s_ba = float(s_b_attn)
    s_wp = float(s_w_proj)
    s_bp = float(s_b_proj)

    # integer-valued dequantized weights; z is 0 for this symmetric scheme
    # so they fit int8 exactly (device casts int8 -> bf16 during the DMA)
    wa_i32 = w_attn_q.astype(np.int32) - int(z_w_attn)
    wp_i32 = w_proj_q.astype(np.int32) - int(z_w_proj)
    assert wa_i32.min() >= -128 and wa_i32.max() <= 127
    assert wp_i32.min() >= -128 and wp_i32.max() <= 127
    wa_int = wa_i32.astype(np.float32)
    wp_int = wp_i32.astype(np.float32)
    wa_i8 = wa_i32.astype(np.int8)
    wp_i8 = wp_i32.astype(np.int8)
    ba_true = s_ba * (b_attn_q.astype(np.int32) - int(z_b_attn)).astype(np.float32)
    bp_true = s_bp * (b_proj_q.astype(np.int32) - int(z_b_proj)).astype(np.float32)

    xT_np = np.ascontiguousarray(
        np.swapaxes(x, 1, 2).reshape(B, CCH, 128, T).transpose(0, 2, 1, 3)
    ).astype(BF16)                                   # [B, 128, CCH, T]

    key = (s_wa,)
    if key not in _CACHE:
        _CACHE[key] = _build_program(s_wa)
    nc = _CACHE[key]

    in_maps = []
    for c in range(NCORES):
        r0 = c * CS                    # q rows for this core's heads
        wq = wa_i8[r0:r0 + CS]                     # [256, C]
        wk = wa_i8[C + r0:C + r0 + CS]
        wv_rows = wa_i8[2 * C + r0:2 * C + r0 + CS]
        wqk_np = np.ascontiguousarray(
            np.concatenate([wq, wk], axis=0).T       # [C, 512]
            .reshape(CCH, 128, 4, 128).transpose(1, 0, 2, 3)
        )                                            # [128, CCH, 4, 128] i8
        wv_np = np.ascontiguousarray(
            wv_rows.T.reshape(CCH, 128, CS).transpose(1, 0, 2)
        )                                            # [128, CCH, 256] i8
        wp_np = np.ascontiguousarray(
            wp_i8[:, r0:r0 + CS].T                   # [256, C]
        ).reshape(HPC, 128, C)
        bq = ba_true[r0:r0 + CS]
        bk = ba_true[C + r0:C + r0 + CS]
        bqk_np = np.ascontiguousarray(
            np.concatenate([bq, bk]).reshape(4, 128).T  # [128, 4]
        ).astype(np.float32)
        in_maps.append({
            "xT": xT_np,
            "wqk0b": wqk_np[:, 0:4].astype(BF16),
            "wqk": wqk_np,
            "wv": wv_np,
            "wp": wp_np,
            "bqk": bqk_np,
        })

    res = run_bass_kernel_spmd(nc, in_maps, core_ids=list(range(NCORES)))

    acc = np.zeros((BT, C), np.float64)
    for c in range(NCORES):
        acc += res.results[c]["partial"].astype(np.float64)
    # v and w_proj were used unscaled on device; apply the commuting scales
    # here. The v-bias passes through normalized attention as a constant row;
    # add it (and the c_proj bias) here, exactly, in fp64->fp32.
    bv_true = ba_true[2 * C:3 * C]
    bv_fold = (s_wp * (bv_true.astype(np.float64) @ wp_int.astype(np.float64).T))
    out = (s_wa * s_wp) * acc + bv_fold[None, :] + bp_true.astype(np.float64)[None, :]
    return out.reshape(B, T, C).astype(np.float32)



# revision 51
# speedup vs baseline: 1.1962x; 1.0030x over previous
"""Trainium2 Bass kernel for quantized causal self-attention.

Sharding: Megatron-style tensor parallelism over heads. 16 heads are split
across 8 NeuronCores (2 heads/core). Each core computes, for all 4 batches:
  - its QKV head-slice projection (x @ w_slice^T, int8 weights held exactly
    in bf16, quant scales folded into epilogues / host),
  - causal attention for its 2 heads (transposed-scores layout, exp without
    max-subtraction - scores are bounded ~5 for this model family),
  - a partial output projection against its column slice of w_proj.
The host sums the 8 partial projections, applies the commuting quant scales
(s_w_attn * s_w_proj), and adds the biases that commute out of the linear
ops (c_proj bias, and the v-bias term which passes through softmax-normalized
attention as a constant row).

Schedule: fully software-pipelined across batches. Every stage s of batch b
interleaves, in PE program order, work with no mutual dependencies:
  scores(b,h0,row s) | QK(b+1, group s) | scores(b,h1,row s) |
  att@v(b,h0,qb=s) | V(b+1, group s) | att@v(b,h1,qb=s) |
  y-transposes | c_proj(b, tb=s-1)
so the tensor engine never waits on the scalar-engine exp or the DVE drain
chains. Batch 3 (no next-batch filler) instead runs its score rows one step
ahead of the avs. The causal-mask multiply runs on the otherwise-idle Pool
engine and the c_proj drains alternate DVE/ACT into one wide staging tile
whose halves ship as two 256KB stores on separate HWDGE rings.

Startup: weights ship int8 (exactly int-valued) and are cast to bf16 by
the SWDGE during the DMA; a chunk-major first QK pass consumes x/wqk
chunks in DMA arrival order so the PE starts ~2us into the HBM burst; wv/
wp/xb1 loads are gated behind 1-element copies of late-produced prologue
data (memset gates get hoisted - the scheduler runs dependency-free
instructions immediately, and each HWDGE ring issues its first ~8 DMAs
back-to-back before becoming completion-paced).
"""

import numpy as np
import ml_dtypes

B, T, C, H, D = 4, 1024, 2048, 16, 128
NCORES = 8
HPC = H // NCORES          # heads per core = 2
CS = HPC * D               # per-core head feature slice = 256
BT = B * T                 # 4096 tokens
TB = T // 128              # 8 token blocks per batch
CCH = C // 128             # 16 contraction chunks

BF16 = ml_dtypes.bfloat16

_CACHE = {}


def _build_program(s_wa: float):
    import concourse.tile as tile
    from concourse import bacc, mybir
    from concourse.masks import make_identity, make_upper_triangular

    f32 = mybir.dt.float32
    bf16 = mybir.dt.bfloat16
    i8 = mybir.dt.int8
    AF = mybir.ActivationFunctionType
    inv_sqrt_d = 1.0 / float(np.sqrt(D))

    nc = bacc.Bacc("TRN2", target_bir_lowering=False, debug=False)

    # all partition-major so SBUF-shaped multi-chunk DMAs need no transpose;
    # wqk is chunk-major so 4-chunk slices are 4KB-contiguous per partition
    # and arrive in the order the chunk-major prologue pass consumes them
    xT = nc.dram_tensor("xT", [B, 128, CCH, T], bf16, kind="ExternalInput")
    # weights ship as int8 (they are exactly int-valued) and are cast to
    # bf16 by the SWDGE during the DMA - halves the critical HBM bytes
    wqk = nc.dram_tensor("wqk", [128, CCH, 4, 128], i8, kind="ExternalInput")
    wv = nc.dram_tensor("wv", [128, CCH, CS], i8, kind="ExternalInput")
    wp = nc.dram_tensor("wp", [HPC, 128, C], i8, kind="ExternalInput")
    bqk = nc.dram_tensor("bqk", [128, 4], f32, kind="ExternalInput")
    partial = nc.dram_tensor("partial", [BT, C], bf16, kind="ExternalOutput")

    with tile.TileContext(nc) as tc:
        with (
            tc.tile_pool(name="singles", bufs=1) as singles,
            tc.tile_pool(name="xpool", bufs=2) as xpool,
            tc.tile_pool(name="qkpool", bufs=3) as qkpool,
            tc.tile_pool(name="vpool", bufs=3) as vpool,
            tc.tile_pool(name="attpool", bufs=3) as attpool,
            tc.tile_pool(name="ytpool", bufs=4) as ytpool,
            tc.tile_pool(name="ypool", bufs=3) as ypool,
            tc.tile_pool(name="rlpool", bufs=4) as rlpool,
            tc.tile_pool(name="outpool", bufs=4) as outpool,
            # 5 banks shared round-robin by the QK / V / c_proj / score
            # matmul groups: a 5-deep ring keeps the PE ~4 groups ahead of
            # the slowest drain (the scalar-engine exp)
            tc.tile_pool(name="psbig", bufs=5, space="PSUM") as psbig,
            # att@v accumulators: freed right after the normalize-multiply
            tc.tile_pool(name="pssm", bufs=2, space="PSUM") as pssm,
            # y-transpose target (PE transpose must land in PSUM)
            tc.tile_pool(name="pstp", bufs=1, space="PSUM") as pstp,
        ):
            # weights / constants, resident for the whole kernel
            wqk_s = singles.tile([128, CCH, 4, 128], bf16, tag="wqk")
            wv_s = singles.tile([128, CCH, CS], bf16, tag="wv")
            wp_s = [singles.tile([128, C], bf16, tag=f"wp{h}", name=f"wp_s{h}")
                    for h in range(HPC)]
            bqk_s = singles.tile([128, 4], f32, tag="bqk")
            trimask = singles.tile([128, 128], bf16, tag="trimask")
            ident = singles.tile([128, 128], bf16, tag="ident")

            # ---- emitters --------------------------------------------------
            def emit_qk_group(xb, qkT, idx, order=2):
                # idx -> (ob, th); ob: 0 = q head0, 1 = q head1, 2 = k
                # head0, 3 = k head1; th: halves of the 1024 tokens.
                # order=1 walks th-major (prologue: matches DMA arrival)
                if order == 2:
                    ob, th = idx // 2, idx % 2
                else:
                    ob, th = idx % 4, idx // 4
                ps = psbig.tile([128, 512], f32, tag="ps", name="ps")
                for g in range(CCH):
                    nc.tensor.matmul(
                        ps[:],
                        wqk_s[:, g, ob, :],
                        xb[:, g, th * 512:(th + 1) * 512],
                        start=(g == 0),
                        stop=(g == CCH - 1),
                    )
                nc.vector.tensor_scalar(
                    qkT[:, ob, th * 512:(th + 1) * 512],
                    ps[:],
                    s_wa,
                    bqk_s[:, ob:ob + 1],
                    mybir.AluOpType.mult,
                    mybir.AluOpType.add,
                )

            def emit_v_group(xb, v_all, tb):
                ps = psbig.tile([128, CS], f32, tag="ps", name="ps")
                for g in range(CCH):
                    nc.tensor.matmul(
                        ps[:],
                        xb[:, g, tb * 128:(tb + 1) * 128],
                        wv_s[:, g, :],
                        start=(g == 0),
                        stop=(g == CCH - 1),
                    )
                nc.vector.tensor_copy(
                    v_all[:, tb, :, 0:D],
                    ps[:].rearrange("p (h d) -> p h d", h=HPC),
                )

            def emit_score_row(qkT, h, attT, kb):
                width = T - kb * 128
                off = 0
                while off < width:
                    w = min(512, width - off)
                    ps = psbig.tile([128, 512], f32, tag="ps", name="ps")
                    nc.tensor.matmul(
                        ps[:, 0:w],
                        qkT[:, 2 + h, kb * 128:(kb + 1) * 128],
                        qkT[:, h, kb * 128 + off:kb * 128 + off + w],
                    )
                    nc.scalar.activation(
                        attT[:, kb, off:off + w],
                        ps[:, 0:w],
                        AF.Exp,
                        scale=inv_sqrt_d,
                    )
                    off += w
                # causal mask on the diagonal block (multiplicative), on the
                # otherwise-idle Pool engine
                nc.gpsimd.tensor_mul(
                    attT[:, kb, 0:128], attT[:, kb, 0:128], trimask[:],
                )

            def emit_av(attT, v_all, h, qb):
                # cols 0:D = y accum, col D = row-sum (ones column of v)
                psy = pssm.tile([128, 256], f32, tag="psy", name="psy")
                for kb in range(qb + 1):
                    nc.tensor.matmul(
                        psy[:, 0:D + 1],
                        attT[:, kb, (qb - kb) * 128:(qb - kb) * 128 + 128],
                        v_all[:, kb, h, :],
                        start=(kb == 0),
                        stop=(kb == qb),
                    )
                rl = rlpool.tile([128, 1], f32, tag="rl", name="rl")
                nc.vector.reciprocal(rl[:], psy[:, D:D + 1])
                ysb = ypool.tile([128, 128], bf16, tag="ysb", name="ysb")
                nc.vector.tensor_scalar_mul(ysb[:], psy[:, 0:D], rl[:])
                return psy, ysb

            def emit_transposes(chs, yTs, qb, pe=True):
                # dedicated PSUM targets (both heads in one bank slot) so
                # psy recycles as soon as the normalize-multiply has read it
                pstile = pstp.tile([128, 128], f32, tag="pst", name="pstile")
                for h, (psy, ysb) in enumerate(chs):
                    pst = pstile[:, 64 * h:64 * h + 64].bitcast(bf16)
                    nc.tensor.transpose(pst, ysb[:], ident[:])
                    nc.vector.tensor_copy(
                        yTs[h][:, qb * 128:(qb + 1) * 128], pst)

            def emit_proj(t0, yTs, tb, obs=(0, 1, 2, 3), po=None):
                # ob 0/1 drain on DVE, ob 2/3 on ACT, so the engine queues
                # stay balanced. Each engine-pair's half of the wide po tile
                # ships as its own 256KB DMA (2KB-contiguous descriptors) as
                # soon as both its drains are in, so the two store receipts
                # overlap at the end of the kernel.
                if po is None:
                    po = outpool.tile([128, C], bf16, tag="po", name="po")
                rows = partial[t0 + tb * 128:t0 + (tb + 1) * 128]
                for ob in obs:
                    ps = psbig.tile([128, 512], f32, tag="ps", name="ps")
                    for h in range(HPC):
                        nc.tensor.matmul(
                            ps[:],
                            yTs[h][:, tb * 128:(tb + 1) * 128],
                            wp_s[h][:, ob * 512:(ob + 1) * 512],
                            start=(h == 0),
                            stop=(h == HPC - 1),
                        )
                    if ob < 2:
                        nc.vector.tensor_copy(
                            po[:, ob * 512:(ob + 1) * 512], ps[:])
                    else:
                        nc.scalar.copy(
                            po[:, ob * 512:(ob + 1) * 512], ps[:])
                    if ob == 1:
                        nc.sync.dma_start(rows[:, 0:1024], po[:, 0:1024])
                    elif ob == 3:
                        # other HWDGE ring, so the two halves' issue and
                        # completion receipts overlap
                        nc.scalar.dma_start(rows[:, 1024:2048],
                                            po[:, 1024:2048])
                return po

            # ---- prologue --------------------------------------------------
            # weights dispatch from the (idle) ACT hardware-DGE ring, x from
            # SP; both rings drain FIFO, so issue in consumption order. The
            # first QK group (ob=0) starts as soon as bqk + wqk[0] (ACT ring)
            # and the first 2-chunk slice of x (SP ring) have landed (~1MB),
            # with the per-chunk matmuls streaming behind the x DMAs.
            xbs = [None] * B
            xbs[0] = xpool.tile([128, CCH, T], bf16, tag="xb", name="xb")
            xbs[1] = xpool.tile([128, CCH, T], bf16, tag="xb", name="xb")
            nc.scalar.dma_start(bqk_s[:], bqk[:])
            # weights ride the SWDGE (Pool) queue with an int8->bf16 cast;
            # x batch 0 alternates across both HWDGE rings (SP + ACT, both
            # idle in the prologue) - th=0 halves first (what the
            # chunk-major pass eats), then th=1. Each ring's first ~8 DMAs
            # issue immediately; later ones are completion-paced, which is
            # exactly the priority order we want. wv/wp/xb1 are gated
            # behind real prologue data (below).
            for q in range(4):
                nc.gpsimd.dma_start(wqk_s[:, 4 * q:4 * q + 4],
                                    wqk[:, 4 * q:4 * q + 4])
            for th in range(2):
                for a in range(8):
                    eng = nc.sync if a % 2 == 0 else nc.scalar
                    eng.dma_start(
                        xbs[0][:, 2 * a:2 * a + 2, th * 512:(th + 1) * 512],
                        xT[0, :, 2 * a:2 * a + 2, th * 512:(th + 1) * 512])
            # valid (1.0) where q >= k for the transposed [k, q] diag block
            make_upper_triangular(nc, trimask[:], val=1.0, diag=True)
            make_identity(nc, ident[:])

            qkTs = [None] * B
            v_alls = [None] * B

            def alloc_batch(b):
                qkTs[b] = qkpool.tile([128, 4, T], bf16, tag="qkT",
                                      name="qkT")
                v_alls[b] = vpool.tile([128, TB, HPC, D + 1], bf16, tag="v",
                                       name="v_all")
                nc.vector.memset(v_alls[b][:, :, :, D:D + 1], 1.0)

            with nc.named_scope("prologue"):
                alloc_batch(0)
                # th=0 chunk-major pass: all 4 ob accumulators advance one
                # chunk at a time, so the PE consumes x/wqk chunks in DMA
                # arrival order at ~the HBM line rate, starting as soon as
                # the first chunk lands instead of after the full 4MB
                pss = [psbig.tile([128, 512], f32, tag="ps", name="ps")
                       for _ in range(4)]
                for g in range(CCH):
                    for ob in range(4):
                        nc.tensor.matmul(
                            pss[ob][:],
                            wqk_s[:, g, ob, :],
                            xbs[0][:, g, 0:512],
                            start=(g == 0),
                            stop=(g == CCH - 1),
                        )
                for ob in range(4):
                    nc.vector.tensor_scalar(
                        qkTs[0][:, ob, 0:512],
                        pss[ob][:],
                        s_wa,
                        bqk_s[:, ob:ob + 1],
                        mybir.AluOpType.mult,
                        mybir.AluOpType.add,
                    )
                # bandwidth gates: a 1-element copy whose SOURCE is data the
                # prologue produces late (a qkT drain) makes the gated DMA
                # genuinely unready until then - the scheduler hoists
                # dependency-free gates like memsets, but not these. The
                # copied garbage is overwritten by the DMA itself.
                nc.vector.tensor_copy(wv_s[:, 0:1, 0:1],
                                      qkTs[0][:, 0:1, 0:1])
                for a in range(2):
                    nc.gpsimd.dma_start(wv_s[:, 8 * a:8 * a + 8, :],
                                        wv[:, 8 * a:8 * a + 8, :])
                nc.vector.tensor_copy(xbs[1][:, 0:1, 0:1],
                                      qkTs[0][:, 1:2, 0:1])
                for a in range(8):
                    nc.sync.dma_start(xbs[1][:, 2 * a:2 * a + 2, :],
                                      xT[1, :, 2 * a:2 * a + 2, :])
                emit_qk_group(xbs[0], qkTs[0], 1)   # (ob0, th1)
                emit_qk_group(xbs[0], qkTs[0], 3)   # (ob1, th1)
                for h in range(HPC):
                    # gated on the (ob0, th1) drain just above
                    nc.vector.tensor_copy(wp_s[h][:, 0:1],
                                          qkTs[0][:, 0:1, 512:513])
                for h in range(HPC):
                    nc.gpsimd.dma_start(wp_s[h][:], wp[h])
                emit_qk_group(xbs[0], qkTs[0], 5)   # (ob2, th1)
                emit_qk_group(xbs[0], qkTs[0], 7)   # (ob3, th1)
                for tb in range(TB):
                    emit_v_group(xbs[0], v_alls[0], tb)

            # ---- pipelined batches ----------------------------------------
            prev = None            # deferred (t0, yTs) projection hand-off
            for b in range(B):
                t0 = b * T
                attTs = [attpool.tile([128, TB, T], bf16, tag="attT",
                                      name=f"attT{h}") for h in range(HPC)]
                yTs = [ytpool.tile([128, T], bf16, tag="yT",
                                   name=f"yT{h}") for h in range(HPC)]
                if b + 1 < B:
                    alloc_batch(b + 1)
                if b + 2 < B:
                    xbs[b + 2] = xpool.tile([128, CCH, T], bf16, tag="xb",
                                            name="xb")

                with nc.named_scope(f"batch{b}"):
                    if b == 3:
                        # no next-batch QK/V filler here, so run the score
                        # rows one step ahead of the avs: row s+1's exp+mask
                        # chain hides under av(s)+proj(s-1) PE work
                        emit_score_row(qkTs[b], 0, attTs[0], 0)
                        emit_score_row(qkTs[b], 1, attTs[1], 0)
                        for s in range(TB):
                            if s + 1 < TB:
                                emit_score_row(qkTs[b], 0, attTs[0], s + 1)
                            if s == 0 and prev is not None:
                                emit_proj(*prev, TB - 1)
                            if s + 1 < TB:
                                emit_score_row(qkTs[b], 1, attTs[1], s + 1)
                            if s > 0:
                                po3 = emit_proj(t0, yTs, s - 1, obs=(2, 3))
                            ch0 = emit_av(attTs[0], v_alls[b], 0, s)
                            ch1 = emit_av(attTs[1], v_alls[b], 1, s)
                            if s > 0:
                                emit_proj(t0, yTs, s - 1, obs=(0, 1), po=po3)
                            emit_transposes((ch0, ch1), yTs, s,
                                            pe=(s == TB - 1))
                        # interleave the final drains DVE/ACT so both
                        # engines finish right after the last matmul
                        emit_proj(t0, yTs, TB - 1, obs=(0, 2, 1, 3))
                    else:
                        for s in range(TB):
                            emit_score_row(qkTs[b], 0, attTs[0], s)
                            emit_qk_group(xbs[b + 1], qkTs[b + 1], s)
                            emit_score_row(qkTs[b], 1, attTs[1], s)
                            if s == 0 and prev is not None:
                                # previous batch's last projection block
                                # lands here, where its y-transpose chain
                                # has surely drained
                                emit_proj(*prev, TB - 1)
                            ch0 = emit_av(attTs[0], v_alls[b], 0, s)
                            emit_v_group(xbs[b + 1], v_alls[b + 1], s)
                            ch1 = emit_av(attTs[1], v_alls[b], 1, s)
                            emit_transposes((ch0, ch1), yTs, s)
                            if s > 0:
                                emit_proj(t0, yTs, s - 1)
                            if b + 2 < B:
                                nc.sync.dma_start(
                                    xbs[b + 2][:, 2 * s:2 * s + 2, :],
                                    xT[b + 2, :, 2 * s:2 * s + 2, :])
                        prev = (t0, yTs)       # proj(b, 7) deferred to b+1

    nc.compile()
    return nc


def kernel(x, w_attn_q, s_w_attn, z_w_attn, b_attn_q, s_b_attn, z_b_attn,
           w_proj_q, s_w_proj, z_w_proj, b_proj_q, s_b_proj, z_b_proj):
    from concourse.bass_utils import run_bass_kernel_spmd

    x = np.asarray(x, np.float32)
    w_attn_q = np.asarray(w_attn_q)
    b_attn_q = np.asarray(b_attn_q)
    w_proj_q = np.asarray(w_proj_q)
    b_proj_q = np.asarray(b_proj_q)
    s_wa = float(s_w_attn)
    s_ba = float(s_b_attn)
    s_wp = float(s_w_proj)
    s_bp = float(s_b_proj)

    # integer-valued dequantized weights; z is 0 for this symmetric scheme
    # so they fit int8 exactly (device casts int8 -> bf16 during the DMA)
    wa_i32 = w_attn_q.astype(np.int32) - int(z_w_attn)
    wp_i32 = w_proj_q.astype(np.int32) - int(z_w_proj)
    assert wa_i32.min() >= -128 and wa_i32.max() <= 127
    assert wp_i32.min() >= -128 and wp_i32.max() <= 127
    wa_int = wa_i32.astype(np.float32)
    wp_int = wp_i32.astype(np.float32)
    wa_i8 = wa_i32.astype(np.int8)
    wp_i8 = wp_i32.astype(np.int8)
    ba_true = s_ba * (b_attn_q.astype(np.int32) - int(z_b_attn)).astype(np.float32)
    bp_true = s_bp * (b_proj_q.astype(np.int32) - int(z_b_proj)).astype(np.float32)

    xT_np = np.ascontiguousarray(
        np.swapaxes(x, 1, 2).reshape(B, CCH, 128, T).transpose(0, 2, 1, 3)
    ).astype(BF16)                                   # [B, 128, CCH, T]

    key = (s_wa,)
    if key not in _CACHE:
        _CACHE[key] = _build_program(s_wa)
    nc = _CACHE[key]

    in_maps = []
    for c in range(NCORES):
        r0 = c * CS                    # q rows for this core's heads
        wq = wa_i8[r0:r0 + CS]                     # [256, C]
        wk = wa_i8[C + r0:C + r0 + CS]
        wv_rows = wa_i8[2 * C + r0:2 * C + r0 + CS]
        wqk_np = np.ascontiguousarray(
            np.concatenate([wq, wk], axis=0).T       # [C, 512]
            .reshape(CCH, 128, 4, 128).transpose(1, 0, 2, 3)
        )                                            # [128, CCH, 4, 128] i8
        wv_np = np.ascontiguousarray(
            wv_rows.T.reshape(CCH, 128, CS).transpose(1, 0, 2)
        )                                            # [128, CCH, 256] i8
        wp_np = np.ascontiguousarray(
            wp_i8[:, r0:r0 + CS].T                   # [256, C]
        ).reshape(HPC, 128, C)
        bq = ba_true[r0:r0 + CS]
        bk = ba_true[C + r0:C + r0 + CS]
        bqk_np = np.ascontiguousarray(
            np.concatenate([bq, bk]).reshape(4, 128).T  # [128, 4]
        ).astype(np.float32)
        in_maps.append({
            "xT": xT_np,
            "wqk0b": wqk_np[:, 0:4].astype(BF16),
            "wqk": wqk_np,
            "wv": wv_np,
            "wp": wp_np,
            "bqk": bqk_np,
        })

    res = run_bass_kernel_spmd(nc, in_maps, core_ids=list(range(NCORES)))

    acc = np.zeros((BT, C), np.float64)
    for c in range(NCORES):
        acc += res.results[c]["partial"].astype(np.float64)
    # v and w_proj were used unscaled on device; apply the commuting scales
    # here. The v-bias passes through normalized attention as a constant row;
    # add it (and the c_proj bias) here, exactly, in fp64->fp32.
    bv_true = ba_true[2 * C:3 * C]
    bv_fold = (s_wp * (bv_true.astype(np.float64) @ wp_int.astype(np.float64).T))
    out = (s_wa * s_wp) * acc + bv_fold[None, :] + bp_true.astype(np.float64)[None, :]
    return out.reshape(B, T, C).astype(np.float32)

